# revision 2
# baseline (speedup 1.0000x reference)
"""Causal multi-head self-attention on 8 Trainium2 NeuronCores — v5.

Sharding: 8 cores = (batch b in 0..3) x (head-half hh in 0..1), as baseline.
Host sums the two per-batch partial outputs (the out-proj all-reduce).

Numerics: the reference multiplies scores by sqrt(head_dim), saturating the
softmax; near-tied rows flip their argmax under small score noise.  fp16
anywhere in the Q/K chain measurably fails (rel ~2.2e-2), so x, Wq, Wk, the
Q^T/K^T activations and the score matmuls all stay float32r (PE fp22 path,
rel ~1.0e-2 — validated against an fp64 CPU simulation of this exact rounding
chain).  The V/P/ctx/Wo path runs fp16 (better than the baseline's bf16).

Performance structure vs the 428us baseline:
 - Phase 1 (Q/K proj) is k-major in 2-ko chunks with chunk-gated DMA issued
   in consumption order: the PE starts ~4us in instead of ~30us.
 - The fp16 V-projection matmuls are woven between the fp32 projection
   matmuls (one V matmul per weight tile pair) so the exposed fp32
   LDWEIGHTS tail overlaps fp16 streaming work.
 - Attention is software-pipelined one unit (head, 512-wide q-group) deep:
   scores for unit u+1 issue before the P-consuming transposes/PV of unit u,
   so the DVE max -> ACT exp chain never stalls the PE, and out-projection
   chunks fill the remaining PE gaps (HAM stays warm).
 - PV matmuls are N=512 over q-groups; P^T staging via PE transposes.

Hardcoded: B=4, S=1024, D=2048, H=16, DH=128, scale = sqrt(128).
"""

from collections import deque

import numpy as np

import concourse.bass as bass
import concourse.tile as tile
from concourse import bacc, mybir
from concourse.bass_utils import run_bass_kernel_spmd

B, S, D = 4, 1024, 2048
H = 16
DH = 128
SCALE = float(DH) ** 0.5
HL = 8          # heads per core
E = HL * DH     # 1024: per-core slice of hidden dim
KO = D // 128   # 16 k-subtiles for d-contraction
ST = S // 128   # 8 sequence tiles
F32 = mybir.dt.float32
F32R = mybir.dt.float32r
FP16 = mybir.dt.float16
NEG = -1.0e30
NCH = 8         # 2-ko DMA chunks


def build_nc():
    nc = bacc.Bacc("TRN2", target_bir_lowering=False, debug=False, num_devices=8)

    xt = nc.dram_tensor("xt", [D, S], F32R, kind="ExternalInput")       # x[b].T
    xtf = nc.dram_tensor("xtf", [D, S], FP16, kind="ExternalInput")     # fp16 copy
    wqt = nc.dram_tensor("wqt", [D, E], F32R, kind="ExternalInput")     # Wq[slice].T
    wkt = nc.dram_tensor("wkt", [D, E], F32R, kind="ExternalInput")
    wvt = nc.dram_tensor("wvt", [D, E], FP16, kind="ExternalInput")
    wot = nc.dram_tensor("wot", [E, D], FP16, kind="ExternalInput")     # Wo[:, slice].T
    maskd = nc.dram_tensor("maskd", [128, 128], F32, kind="ExternalInput")
    identd = nc.dram_tensor("identd", [128, 128], FP16, kind="ExternalInput")
    out = nc.dram_tensor("out", [S, D], F32, kind="ExternalOutput")

    xt_r = xt.ap().rearrange("(ko p) s -> p ko s", p=128)       # [128, 16, 1024]
    xtf_r = xtf.ap().rearrange("(ko p) s -> p ko s", p=128)
    wqt_r = wqt.ap().rearrange("(ko p) e -> p ko e", p=128)
    wkt_r = wkt.ap().rearrange("(ko p) e -> p ko e", p=128)
    wvt_r = wvt.ap().rearrange("(ko p) e -> p ko e", p=128)
    wot_r = wot.ap().rearrange("(eo p) o -> p eo o", p=128)     # [128, 8, 2048]
    out_r = out.ap().rearrange("(so p) o -> p so o", p=128)     # [128, 8, 2048]

    with tile.TileContext(nc) as tc:
        # ---------------- phase-1 pools ----------------
        ph1 = tc.alloc_tile_pool(name="ph1", bufs=8, space="PSUM")

        persist = tc.alloc_tile_pool(name="persist", bufs=1)
        mask_sb = persist.tile([128, 128], F32)
        ident_sb = persist.tile([128, 128], FP16)
        qt_sb = persist.tile([128, HL, S], F32R)    # QT: [dh, head, s]
        kt_sb = persist.tile([128, HL, S], F32R)
        smalls = tc.alloc_tile_pool(name="smalls", bufs=6)

        xbpool = tc.alloc_tile_pool(name="xbpool", bufs=1)
        xbf_sb = xbpool.tile([128, KO, S], FP16)
        wvapool = tc.alloc_tile_pool(name="wvapool", bufs=1)
        wva_sb = wvapool.tile([128, KO, 512], FP16)    # Wv e-half 0
        xtpool = tc.alloc_tile_pool(name="xtpool", bufs=1)
        xt_sb = xtpool.tile([128, KO, S], F32R)
        wqk = tc.alloc_tile_pool(name="wqk", bufs=3)   # f32 weight e-tile slots

        # ---------------- DMA: phase-1 stream, consumption order ----------
        # Two HWDGE queues: the fp32 x/weight stream rides the sync queue in
        # consumption order; the fp16 bulk (xbf, wv) rides the ACT queue so
        # it never delays the weight slots.
        nc.scalar.dma_start(mask_sb[:], maskd.ap())
        nc.scalar.dma_start(ident_sb[:], identd.ap())

        def wslot_dma(src_r, et, nchunk=1):
            slot = wqk.tile([128, KO, 128], F32R, tag="w", name=f"wslot{et}")
            step = KO // nchunk
            for c in range(nchunk):
                nc.sync.dma_start(
                    slot[:, c * step:(c + 1) * step, :],
                    src_r[:, c * step:(c + 1) * step, et * 128:(et + 1) * 128],
                )
            return slot

        # few DMA instructions, interleaved in consumption order: the sync
        # engine issues descriptors at ~1/us, so instruction count gates the
        # pipeline spin-up more than bytes do.
        slots = deque()
        slots.append(wslot_dma(wqt_r, 0, nchunk=2))
        for j in range(NCH):
            nc.sync.dma_start(
                xt_sb[:, 2 * j:2 * j + 2, :], xt_r[:, 2 * j:2 * j + 2, :]
            )
            if j == 2:
                slots.append(wslot_dma(wqt_r, 1, nchunk=2))
            elif j == 5:
                slots.append(wslot_dma(wqt_r, 2))
        # fp16 x and Wv-half-0 pre-stage on the ACT queue during phase 1
        # (virgin SBUF, no release-wait): V-proj can start the moment the
        # projections finish.
        for si in range(ST):
            nc.scalar.dma_start(
                xbf_sb[:, :, si * 128:(si + 1) * 128],
                xtf_r[:, :, si * 128:(si + 1) * 128],
            )
        for c in range(2):
            nc.scalar.dma_start(
                wva_sb[:, 8 * c:8 * (c + 1), :], wvt_r[:, 8 * c:8 * (c + 1), 0:512]
            )

        # ---------------- Phase 1: Q/K projections ----------------
        def qk_sweep(wslot, dst, et_g):
            """e-tile sweep: 2 PSUM accumulators (s-halves), k-major over 8
            chunks so the first matmuls start on the first DMA chunks."""
            acc0 = ph1.tile([128, 512], F32, tag="ph1", name="acc0")
            acc1 = ph1.tile([128, 512], F32, tag="ph1", name="acc1")
            for j in range(NCH):
                for dko in range(2):
                    ko = 2 * j + dko
                    wt = wslot[:, ko, :]
                    nc.tensor.matmul(
                        acc0[:], wt, xt_sb[:, ko, 0:512],
                        start=(ko == 0), stop=(ko == KO - 1),
                        skip_group_check=True,
                    )
                    nc.tensor.matmul(
                        acc1[:], wt, xt_sb[:, ko, 512:1024],
                        start=(ko == 0), stop=(ko == KO - 1),
                        skip_group_check=True,
                    )
            if dst is qt_sb:   # fold the softmax scale into Q^T
                nc.scalar.activation(dst[:, et_g, 0:512], acc0[:],
                                     mybir.ActivationFunctionType.Copy,
                                     scale=SCALE)
                nc.scalar.activation(dst[:, et_g, 512:1024], acc1[:],
                                     mybir.ActivationFunctionType.Copy,
                                     scale=SCALE)
            else:
                nc.scalar.copy(dst[:, et_g, 0:512], acc0[:])
                nc.scalar.copy(dst[:, et_g, 512:1024], acc1[:])

        # 16 sweeps; slot DMA for sweep s+2 emitted after sweep s (bufs=2 WAR).
        for s in range(16):
            cur = slots.popleft()
            dst, et_g = (qt_sb, s) if s < 8 else (kt_sb, s - 8)
            qk_sweep(cur, dst, et_g)
            if s + 3 < 16:
                src_r, et = ((wqt_r, s + 3) if s + 3 < 8
                             else (wkt_r, s + 3 - 8))
                slots.append(wslot_dma(src_r, et))

        # ---------------- attention pools (reuse xt/wqk space) -------------
        ph1.release()
        wqk.release()
        xtpool.release()
        pp = tc.alloc_tile_pool(name="pp", bufs=1, space="PSUM")      # outproj
        ps_s = tc.alloc_tile_pool(name="ps_s", bufs=4, space="PSUM")  # scores
        ps_t = tc.alloc_tile_pool(name="ps_t", bufs=2, space="PSUM")  # transposes
        ps_c = tc.alloc_tile_pool(name="ps_c", bufs=1, space="PSUM")  # ctx accum

        vpool = tc.alloc_tile_pool(name="vpool", bufs=1)
        v_sb = vpool.tile([128, ST, E], FP16)       # V: [s_in, s_out_tile, e]
        ppool = tc.alloc_tile_pool(name="ppool", bufs=8)     # softmax P rows
        ptpool = tc.alloc_tile_pool(name="ptpool", bufs=4)   # transposed P
        ctxpool = tc.alloc_tile_pool(name="ctxpool", bufs=16)  # ctx^T per (g,h)
        stage = tc.alloc_tile_pool(name="stage", bufs=2)     # out staging
        wvbpool = tc.alloc_tile_pool(name="wvbpool", bufs=1)  # top: freed at wo
        wvb_sb = wvbpool.tile([128, KO, 512], FP16)  # Wv e-half 1

        # Wv half 1 reuses freed xt space; sync queue is idle by now.
        for c in range(2):
            nc.sync.dma_start(
                wvb_sb[:, 8 * c:8 * (c + 1), :],
                wvt_r[:, 8 * c:8 * (c + 1), 512:1024],
            )

        # ---------------- helper emitters ----------------
        def vproj_chunk(si, ec):
            wsrc = wva_sb if ec == 0 else wvb_sb
            ps = pp.tile([128, 512], F32, tag="pp", name="vpps")
            for ko in range(KO):
                nc.tensor.matmul(
                    ps[:],
                    xbf_sb[:, ko, si * 128:(si + 1) * 128],
                    wsrc[:, ko, :],
                    start=(ko == 0),
                    stop=(ko == KO - 1),
                )
            nc.scalar.copy(v_sb[:, si, ec * 512:(ec + 1) * 512], ps[:])

        ctxts = {}
        wo_state = {}

        def alloc_wo():
            """Once V-proj is fully emitted: reclaim wv space, load Wo
            chunked by output column group (outproj (si,oc) gates on oc)."""
            wvbpool.release()
            wopool = tc.alloc_tile_pool(name="wopool", bufs=1)
            wo_sb = wopool.tile([128, HL, D], FP16)
            for oc in range(4):
                nc.sync.dma_start(
                    wo_sb[:, :, oc * 512:(oc + 1) * 512],
                    wot_r[:, :, oc * 512:(oc + 1) * 512],
                )
            wo_state["sb"] = wo_sb
            wo_state["pool"] = wopool

        def outproj_chunk(g, si, oc, psum_pool, psum_tag):
            qo = (si - 4 * g) * 128
            pso = psum_pool.tile([128, 512], F32, tag=psum_tag, name="pso")
            for jh in range(HL):
                nc.tensor.matmul(
                    pso[:],
                    ctxts[(g, jh)][:, qo:qo + 128],
                    wo_state["sb"][:, jh, oc * 512:(oc + 1) * 512],
                    start=(jh == 0),
                    stop=(jh == HL - 1),
                )
            ob = stage.tile([128, 512], F32, tag="ob")
            if (si + oc) % 2:
                nc.scalar.copy(ob[:], pso[:])
                nc.sync.dma_start(out_r[:, si, oc * 512:(oc + 1) * 512], ob[:])
            else:
                nc.vector.tensor_copy(ob[:], pso[:])
                nc.scalar.dma_start(out_r[:, si, oc * 512:(oc + 1) * 512],
                                    ob[:])

        def stats_t(h, t, p_dst):
            """Scores + masked scaled softmax for (head h, q-tile t)."""
            W = (t + 1) * 128
            N1 = min(W, 512)
            N2 = W - N1
            qs = qt_sb[:, h, t * 128:(t + 1) * 128]
            ps0 = ps_s.tile([128, 512], F32, tag="s", name="ps0")
            nc.tensor.matmul(ps0[:, :N1], qs, kt_sb[:, h, 0:N1],
                             start=True, stop=True)
            ps1 = None
            if N2:
                ps1 = ps_s.tile([128, 512], F32, tag="s", name="ps1")
                nc.tensor.matmul(ps1[:, :N2], qs, kt_sb[:, h, 512:W],
                                 start=True, stop=True)
            if t < 4:
                diag = ps0[:, t * 128:(t + 1) * 128]
            else:
                diag = ps1[:, (t - 4) * 128:(t - 3) * 128]
            nc.vector.tensor_add(diag, diag, mask_sb[:])

            nm = smalls.tile([128, 1], F32, tag="nm")
            nc.vector.reduce_max(nm[:], ps0[:, :N1], axis=mybir.AxisListType.X,
                                 negate=True)
            if N2:
                nm1 = smalls.tile([128, 1], F32, tag="nm1")
                nc.vector.reduce_max(nm1[:], ps1[:, :N2],
                                     axis=mybir.AxisListType.X, negate=True)
                nc.vector.tensor_tensor(nm[:], nm[:], nm1[:],
                                        mybir.AluOpType.min)
            p_sb = ppool.tile([128, S], FP16, tag="p")
            r0 = smalls.tile([128, 1], F32, tag="r0")
            nc.scalar.activation(
                p_sb[:, :N1], ps0[:, :N1], mybir.ActivationFunctionType.Exp,
                bias=nm[:], scale=1.0, accum_out=r0[:],
            )
            if N2:
                r1 = smalls.tile([128, 1], F32, tag="r1")
                nc.scalar.activation(
                    p_sb[:, 512:W], ps1[:, :N2],
                    mybir.ActivationFunctionType.Exp,
                    bias=nm[:], scale=1.0, accum_out=r1[:],
                )
                nc.vector.tensor_tensor(r0[:], r0[:], r1[:],
                                        mybir.AluOpType.add)
            rr = smalls.tile([128, 1], F32, tag="rr")
            nc.vector.reciprocal(rr[:], r0[:])
            nc.vector.tensor_scalar_mul(p_sb[:, :W], p_sb[:, :W], rr[:])
            p_dst[t] = p_sb

        def body_closures(g, h, p_tiles):
            """P^T transposes + PV + ctx copy for one finished unit."""
            cl = []
            nk = 4 * (g + 1)
            holder = {}

            def mk_transp(j):
                def f():
                    off = max(0, (j - 4 * g) * 128)
                    pt_ps = ps_t.tile([128, 512], FP16, tag="t", name="ptps")
                    for t in range(max(j, 4 * g), 4 * g + 4):
                        col = (t - 4 * g) * 128
                        nc.tensor.transpose(
                            pt_ps[:, col:col + 128],
                            p_tiles[t][:, j * 128:(j + 1) * 128],
                            ident_sb[:],
                        )
                    ptsb = ptpool.tile([128, 512], FP16, tag="pt", name="ptsb")
                    if j % 2 == 0:
                        nc.scalar.copy(ptsb[:, off:], pt_ps[:, off:])
                    else:
                        nc.vector.tensor_copy(ptsb[:, off:], pt_ps[:, off:])
                    holder[j] = ptsb
                return f

            def mk_pv(j):
                def f():
                    off = max(0, (j - 4 * g) * 128)
                    if j == 0:
                        holder["ctx"] = ps_c.tile(
                            [128, 512], F32, tag="c", name="ctxps")
                    ctx_ps = holder["ctx"]
                    ptsb = holder.pop(j)
                    nc.tensor.matmul(
                        ctx_ps[:, off:], v_sb[:, j, h * 128:(h + 1) * 128],
                        ptsb[:, off:],
                        start=(j == 0), stop=(j == nk - 1),
                        skip_group_check=True,
                    )
                return f

            def mk_ctx():
                def f():
                    ctxt = ctxpool.tile([128, 512], FP16, tag="ctx",
                                        name="ctxt")
                    nc.scalar.copy(ctxt[:], holder["ctx"][:])
                    ctxts[(g, h)] = ctxt
                return f

            for j in range(nk):
                cl.append(mk_transp(j))
                cl.append(mk_pv(j))
            cl.append(mk_ctx())
            return cl

        def merge(body, fillers):
            out_l = []
            fi = 0
            step = max(1, len(body) // max(1, len(fillers))) if fillers else 0
            for i, b in enumerate(body):
                out_l.append(b)
                if fillers and fi < len(fillers) and i % step == step - 1:
                    out_l.append(fillers[fi])
                    fi += 1
            out_l.extend(fillers[fi:])
            return out_l

        # ---------------- Phase 2: attention units ----------------
        units = [(0, h) for h in range(HL)] + [(1, h) for h in range(HL)]
        queue = deque([lambda si=si: vproj_chunk(si, 0) for si in range(4)])

        for ui, (g, h) in enumerate(units):
            p_tiles = {}
            drained = 0
            n0 = len(queue)
            for i, t in enumerate(range(4 * g, 4 * g + 4)):
                stats_t(h, t, p_tiles)
                want = (n0 * (i + 1)) // 4
                while drained < want and queue:
                    queue.popleft()()
                    drained += 1
            while queue:
                queue.popleft()()

            body = body_closures(g, h, p_tiles)
            fillers = []
            if 1 <= ui <= 6:   # remaining V-proj chunks spread over g0 units
                # ec1 of s-tiles 0..3 must be emitted before unit (0,4)'s PV
                # (heads >=4 read the upper e-half): front-load them.
                rest = [(0, 1), (1, 1), (2, 1), (3, 1), (4, 0), (5, 0),
                        (6, 0), (7, 0), (4, 1), (5, 1), (6, 1), (7, 1)]
                lo = (12 * (ui - 1)) // 6
                hi = (12 * ui) // 6
                fillers = [lambda si=si, ec=ec: vproj_chunk(si, ec)
                           for si, ec in rest[lo:hi]]
            elif 9 <= ui <= 15:  # out-proj q-tiles 0..3 spread over g1 units
                chunks = [(si, oc) for si in range(4) for oc in range(4)]
                lo = (16 * (ui - 9)) // 7
                hi = (16 * (ui - 8)) // 7
                fillers = [
                    lambda si=si, oc=oc: outproj_chunk(0, si, oc, pp, "pp")
                    for si, oc in chunks[lo:hi]
                ]
            queue.extend(merge(body, fillers))
            if ui == 8:
                alloc_wo()

        # tail: drain last unit's body, then out-proj of q-tiles 4..7
        while queue:
            queue.popleft()()
        for k, (si, oc) in enumerate(
                [(si, oc) for si in range(4, 8) for oc in range(4)]):
            outproj_chunk(1, si, oc, pp if k % 4 == 3 else ps_s,
                          "pp" if k % 4 == 3 else "s")

        for p in (wo_state["pool"], stage, ctxpool, ptpool, ppool, vpool,
                  wvapool, xbpool, smalls, persist, ps_c, ps_t, ps_s, pp):
            p.release()

    nc.compile()
    return nc


_NC = None


def _get_nc():
    global _NC
    if _NC is None:
        _NC = build_nc()
    return _NC


def _make_in_maps(x, Wq, Wk, Wv, Wo):
    x = np.asarray(x, dtype=np.float32)
    Wq = np.asarray(Wq, dtype=np.float32)
    Wk = np.asarray(Wk, dtype=np.float32)
    mask = np.triu(np.full((128, 128), NEG, dtype=np.float32), k=1)
    ident = np.eye(128, dtype=np.float16)

    xts = [np.ascontiguousarray(x[b].T) for b in range(B)]
    xtfs = [t.astype(np.float16) for t in xts]
    wqts = [np.ascontiguousarray(Wq[hh * E:(hh + 1) * E, :].T)
            for hh in range(2)]
    wkts = [np.ascontiguousarray(Wk[hh * E:(hh + 1) * E, :].T)
            for hh in range(2)]
    wvts = [np.ascontiguousarray(
                np.asarray(Wv, np.float32)[hh * E:(hh + 1) * E, :].T
            ).astype(np.float16) for hh in range(2)]
    wots = [np.ascontiguousarray(
                np.asarray(Wo, np.float32)[:, hh * E:(hh + 1) * E].T
            ).astype(np.float16) for hh in range(2)]

    in_maps = []
    for b in range(B):
        for hh in range(2):
            in_maps.append({
                "xt": xts[b],
                "xtf": xtfs[b],
                "wqt": wqts[hh],
                "wkt": wkts[hh],
                "wvt": wvts[hh],
                "wot": wots[hh],
                "maskd": mask,
                "identd": ident,
            })
    return in_maps


def run(x, Wq, Wk, Wv, Wo, **rb_kwargs):
    """Run on 8 cores; returns (output [B,S,D], BassKernelResults)."""
    nc = _get_nc()
    in_maps = _make_in_maps(x, Wq, Wk, Wv, Wo)
    res = run_bass_kernel_spmd(nc, in_maps, core_ids=list(range(8)), **rb_kwargs)
    out = np.empty((B, S, D), dtype=np.float32)
    for b in range(B):
        out[b] = res.results[2 * b]["out"] + res.results[2 * b + 1]["out"]
    return out, res


def kernel(x, Wq, Wk, Wv, Wo):
    out, _ = run(x, Wq, Wk, Wv, Wo)
    return out
